# revision 29
# baseline (speedup 1.0000x reference)
"""Trainium2 Bass kernel for nn_CriticUAVob (attention-pool critic).

Math per batch item: two attention-pool branches over s [N=128, 3], then a
tiny MLP.  With X = [x, 1] ([128, 4] augmented), A_b = Wq' Wk'^T/4,
U = exp(X A_b X^T), Z[n] = sum_m U[n, m]:

    pooled_b = (Wv'^T t)/N,   t[k] = sum_n (1/Z[n]) sum_m U[n, m] X[m, k]

Device dataflow per quad of 4 items (U^T layout: m on partitions):
  st [128(m), (i,b,n)] = sTq^T @ btY_bd   (host-prepped block-diag rhs)
  ut = exp(st)                            ScalarE, PSUM -> SBUF bf16
  G [128(n), 4] per (i,b) = ut_blk^T @ xon_i   (k = [1,x,y,z], Z at k=0)
  r [128, 8] = 1/Z ; gw = G * r           wide DVE ops only
  t [128, 1] = gw^T @ ones                one matmul; partitions 32i+4b+k
MLP stage 1 contracts (b, k) per i-strip (32-aligned), so quads are formed
as items {q, 128+q, 256+q, 384+q} to make output columns land in order.
All transposes / X@A products / Wv-W1 folds precomputed on host; b3 is
added on host after gather.

Sharding: pure data parallel, batch split across 8 NeuronCores.
"""
import os
import sys
import numpy as np

sys.path.insert(0, "/opt/trn_rl_repo")

import concourse.bass as bass
import concourse.tile as tile
from concourse import bacc, mybir
from concourse import bass_utils
import ml_dtypes

N_CORES = 8
B = 4096
N = 128
BC = B // N_CORES          # 512 items per core
QUADS = BC // 4            # 128 groups of 4 items
F32 = mybir.dt.float32
BF16 = mybir.dt.bfloat16
AF = mybir.ActivationFunctionType

_cache = {}


def _build():
    nc = bacc.Bacc(
        "TRN2",
        target_bir_lowering=False,
        debug=False,
        enable_asserts=False,
        num_devices=N_CORES,
    )
    sT_t = nc.dram_tensor("sT", [16, 128 * QUADS], BF16, kind="ExternalInput")
    xon_t = nc.dram_tensor("xon", [128, 16 * QUADS], BF16, kind="ExternalInput")
    bty_t = nc.dram_tensor("bty", [QUADS, 16, 1024], BF16, kind="ExternalInput")
    cc_t = nc.dram_tensor("cc", [128, 2048], BF16, kind="ExternalInput")
    w2_t = nc.dram_tensor("w2", [128, 128], BF16, kind="ExternalInput")
    w3_t = nc.dram_tensor("w3", [128, 1], BF16, kind="ExternalInput")
    b1_t = nc.dram_tensor("b1", [128, 1], F32, kind="ExternalInput")
    b2_t = nc.dram_tensor("b2", [128, 1], F32, kind="ExternalInput")
    out_t = nc.dram_tensor("out", [BC, 1], F32, kind="ExternalOutput")

    with tile.TileContext(nc) as tc:
        with (
            tc.tile_pool(name="singles", bufs=1) as singles,
            tc.tile_pool(name="btyp", bufs=3) as btyp,
            tc.tile_pool(name="utp", bufs=4) as utp,
            tc.tile_pool(name="smallp", bufs=3) as smallp,
            tc.tile_pool(name="pst", bufs=3, space="PSUM") as pst,
            tc.tile_pool(name="psg", bufs=1, space="PSUM") as psg,
            tc.tile_pool(name="ptt", bufs=1, space="PSUM") as ptt,
        ):
            # s-data in 4 chunks so quad 0 starts after ~128 KB, not ~1.5 MB
            sT_c0 = singles.tile([16, 4096], BF16)
            sT_c1 = singles.tile([16, 4096], BF16)
            sT_c2 = singles.tile([16, 4096], BF16)
            sT_c3 = singles.tile([16, 4096], BF16)
            sTcs = [sT_c0, sT_c1, sT_c2, sT_c3]
            xon_c0 = singles.tile([128, 512], BF16)
            xon_c1 = singles.tile([128, 512], BF16)
            xon_c2 = singles.tile([128, 512], BF16)
            xon_c3 = singles.tile([128, 512], BF16)
            xoncs = [xon_c0, xon_c1, xon_c2, xon_c3]
            for c in range(4):
                nc.sync.dma_start(
                    sTcs[c][:], sT_t.ap()[:, c * 4096:(c + 1) * 4096]
                )
                nc.sync.dma_start(
                    xoncs[c][:], xon_t.ap()[:, c * 512:(c + 1) * 512]
                )

            ones = singles.tile([128, 1], BF16)
            nc.gpsimd.memset(ones[:], 1.0)
            # t accumulator: rows 32i+8p+4b+k, cols = quad-group (4 quads)
            tbig = singles.tile([128, QUADS // 4], BF16)
            # gw holds 4 quads (p = q%4) before each t-matmul; double buffered
            gw_a = singles.tile([128, 128], BF16)
            gw_b = singles.tile([128, 128], BF16)
            gws = [gw_a, gw_b]
            nc.gpsimd.memset(gw_a[:], 0.0)
            nc.gpsimd.memset(gw_b[:], 0.0)

            bty_ap = bty_t.ap()
            cc = w2 = w3 = b1 = b2 = None

            for q in range(QUADS):
                bty = btyp.tile([16, 1024], BF16, tag="bty")
                nc.sync.dma_start(bty[:], bty_ap[q])

                if q == 8:
                    # MLP weights: issued mid-loop while DMA queues are idle
                    cc = singles.tile([128, 2048], BF16)
                    nc.sync.dma_start(cc[:], cc_t.ap())
                    w2 = singles.tile([128, 128], BF16)
                    nc.sync.dma_start(w2[:], w2_t.ap())
                    w3 = singles.tile([128, 1], BF16)
                    nc.sync.dma_start(w3[:], w3_t.ap())
                    b1 = singles.tile([128, 1], F32)
                    nc.sync.dma_start(b1[:], b1_t.ap())
                    b2 = singles.tile([128, 1], F32)
                    nc.sync.dma_start(b2[:], b2_t.ap())

                lhs = sTcs[q // 32][:, (q % 32) * 128:(q % 32 + 1) * 128]
                ps = pst.tile([128, 1024], F32, tag="st")
                nc.tensor.matmul(ps[:, 0:512], lhs, bty[:, 0:512])
                nc.tensor.matmul(ps[:, 512:1024], lhs, bty[:, 512:1024])

                ut = utp.tile([128, 1024], BF16, tag="ut")
                nc.scalar.activation(ut[:], ps[:], AF.Exp)

                # G per (i,b): [128(n), 4(k)] at ps_g col 32i+4b
                ps_g = psg.tile([128, 128], F32, tag="g")
                xonc = xoncs[q // 32]
                xq = (q % 32) * 16
                for i in range(4):
                    xsl = xonc[:, xq + i * 4:xq + (i + 1) * 4]
                    c0 = i * 256
                    nc.tensor.matmul(
                        ps_g[:, 32 * i:32 * i + 4], ut[:, c0:c0 + 128], xsl
                    )
                    nc.tensor.matmul(
                        ps_g[:, 32 * i + 4:32 * i + 8],
                        ut[:, c0 + 128:c0 + 256], xsl,
                    )

                # r = 1/Z (Z at k=0 of each (i,b) block), gw = G * r
                g4 = ps_g[:].rearrange("p (i b k) -> p i b k", i=4, b=8)
                r = smallp.tile([128, 8], F32, tag="r")
                r2 = r[:].rearrange("p (i b) -> p i b", i=4)
                nc.vector.reciprocal(r2, g4[:, :, 0:2, 0])
                gw = gws[(q // 4) % 2]
                gw5 = gw[:].rearrange(
                    "p (i pp b k) -> p i pp b k", i=4, pp=4, b=2
                )[:, :, q % 4, :, :]
                rb = r2.unsqueeze(3).broadcast_to([128, 4, 2, 4])
                nc.vector.tensor_mul(gw5, g4[:, :, 0:2, :], rb)

                # t = sum_n gw -> [128, 1] at partitions 32i+8p+4b+k; 1 per
                # 4 quads
                if q % 4 == 3:
                    ps_t = ptt.tile([128, 1], F32, tag="t")
                    nc.tensor.matmul(ps_t[:], gw[:], ones[:])
                    nc.vector.tensor_copy(tbig[:, q // 4:q // 4 + 1], ps_t[:])

            # ---- batched MLP; item of (qq, p, i) is (p*4+i)*32+qq so the
            # 16 variant matmuls land output columns in natural order
            ps_h = pst.tile([128, BC], F32, tag="st")
            for v in range(16):
                nc.tensor.matmul(
                    ps_h[:, v * 32:(v + 1) * 32],
                    cc[:, v * 128:(v + 1) * 128],
                    tbig[:],
                )
            h1 = singles.tile([128, BC], BF16)
            nc.scalar.activation(h1[:], ps_h[:], AF.Tanh, bias=b1[:])

            ps_z2 = pst.tile([128, BC], F32, tag="st")
            nc.tensor.matmul(ps_z2[:], w2[:], h1[:])
            h2 = singles.tile([128, BC], BF16)
            nc.scalar.activation(h2[:], ps_z2[:], AF.Tanh, bias=b2[:])

            ps_z3 = psg.tile([1, BC], F32, tag="g")
            nc.tensor.matmul(ps_z3[:], w3[:], h2[:])
            y_sb = singles.tile([1, BC], F32)
            nc.vector.tensor_copy(y_sb[:], ps_z3[:])

            nc.sync.dma_start(out_t.ap().rearrange("b o -> o b"), y_sb[:])

    nc.compile()
    return nc


def _host_prep(inputs):
    f = lambda x: np.asarray(x, dtype=np.float32)
    bf = lambda x: np.ascontiguousarray(x).astype(ml_dtypes.bfloat16)
    s_obs = f(inputs["s_obs"])

    def aug(Wk, bk):
        return np.vstack([f(inputs[Wk]), f(inputs[bk]).reshape(1, -1)])

    Wq_rs, Wk_rs = aug("Wq_rs", "bq_rs"), aug("Wk_rs", "bk_rs")
    Wq_tg, Wk_tg = aug("Wq_tg", "bq_tg"), aug("Wk_tg", "bk_tg")
    Wv_rs, Wv_tg = aug("Wv_rs", "bv_rs"), aug("Wv_tg", "bv_tg")

    scale = 1.0 / np.sqrt(16.0)
    A_rs = (Wq_rs @ Wk_rs.T * scale).astype(np.float32)   # [4, 4]
    A_tg = (Wq_tg @ Wk_tg.T * scale).astype(np.float32)

    W1 = f(inputs["W1"])                                   # [64, 128]
    # cc rows 4b+k (replicated per 32-strip): C_b[k] = (Wv_b[k-1]/N) @ W1blk
    CC = np.zeros((8, 128), np.float32)
    CC[1:4] = (Wv_rs[0:3] / N) @ W1[0:32]
    CC[5:8] = (Wv_tg[0:3] / N) @ W1[32:64]
    # cc variant v = p*4+i: nonzero only at partitions 32i+8p+(0..7)
    cc = np.zeros((128, 2048), np.float32)
    for v in range(16):
        p, i = v // 4, v % 4
        cc[32 * i + 8 * p:32 * i + 8 * p + 8, v * 128:(v + 1) * 128] = CC
    b1_eff = (f(inputs["b1"]) + Wv_rs[3] @ W1[0:32] + Wv_tg[3] @ W1[32:64])

    common = dict(
        cc=bf(cc),
        w2=bf(f(inputs["W2"])), w3=bf(f(inputs["W3"])),
        b1=b1_eff.reshape(128, 1).astype(np.float32),
        b2=f(inputs["b2"]).reshape(128, 1),
    )

    # quad q = (qq, p): position (q, i) holds item (p*4+i)*32 + qq
    order = np.empty(BC, dtype=np.int64)
    for q in range(QUADS):
        p, qq = q % 4, q // 4
        for i in range(4):
            order[q * 4 + i] = (p * 4 + i) * 32 + qq

    in_maps = []
    for c in range(N_CORES):
        s_c = s_obs[c * BC:(c + 1) * BC][order]            # [512, 128, 3]
        Xa = np.concatenate([s_c, np.ones((BC, N, 1), np.float32)], axis=2)

        # sT_all [16, QUADS*128]: rows (i, j) j-order [x, y, z, 1]
        sT = Xa.reshape(QUADS, 4, N, 4).transpose(1, 3, 0, 2).reshape(16, -1)

        # xon_all [128, QUADS*16]: cols (q, i, k) k-order [1, x, y, z]
        Xon = np.concatenate([np.ones((BC, N, 1), np.float32), s_c], axis=2)
        xon = Xon.transpose(1, 0, 2).reshape(N, -1)

        # btY block-diag [QUADS, 16, 1024]: diag block i = Y^T [4, (br, n)]
        Y = np.stack([Xa @ A_rs, Xa @ A_tg], axis=1)       # [512, 2, 128, 4]
        blocks = Y.transpose(0, 3, 1, 2).reshape(BC, 4, 256)
        bd = np.zeros((QUADS, 16, 1024), np.float32)
        bdv = bd.reshape(QUADS, 4, 4, 4, 256)
        blv = blocks.reshape(QUADS, 4, 4, 256)
        for i in range(4):
            bdv[:, i, :, i, :] = blv[:, i]

        m = dict(common)
        m["sT"] = bf(sT)
        m["xon"] = bf(xon)
        m["bty"] = bf(bd)
        in_maps.append(m)
    return in_maps


def kernel(**inputs):
    if "nc" not in _cache:
        _cache["nc"] = _build()
    nc = _cache["nc"]
    in_maps = _host_prep(inputs)
    trace = os.environ.get("KERNEL_TRACE", "0") == "1"
    res = bass_utils.run_bass_kernel_spmd(
        nc, in_maps, core_ids=list(range(N_CORES)), trace=trace
    )
    _cache["last"] = res
    b3 = float(np.asarray(inputs["b3"]).reshape(-1)[0])
    out = np.concatenate([r["out"] for r in res.results], axis=0) + b3
    return out.astype(np.float32)
